# revision 1
# baseline (speedup 1.0000x reference)
"""Trainium2 Bass kernel for nn_Attention_27994596836196.

GQA attention block (B=2, S=2048, HID=4096, 32 q heads / 8 kv groups,
rope, causal, out-projection), tensor-parallel over the 8 NeuronCores of
one TRN2 chip: core c owns q heads 4c..4c+3 and kv group c.  Inside the
device kernel each core computes its heads' Q^T/K^T/V projections from a
host-pretransposed activation matrix, runs causal flash-style attention
in a transposed (keys-on-partitions) layout, and contracts its 512-row
slice of w_o into a full-size partial output; the host sums the eight
partials (collectives deliberately avoided: a collective in the NEFF
measurably slows every PE instruction by ~20%).

Self-contained: builds and runs via concourse (bass/tile) from
/opt/trn_rl_repo through bass_utils.run_bass_kernel_spmd on cores 0-7.
"""

import os
import sys

sys.path.insert(0, "/opt/trn_rl_repo")

import numpy as np
import ml_dtypes

# NTFF profiling hook shim: this agent image's antenv package lacks
# axon_hooks, which run_bass_kernel_spmd(trace=True) imports.  Harmless
# when tracing is off; registers the real hook when available.
try:
    import antenv.axon_hooks  # noqa: F401
except ImportError:
    import types

    _m = types.ModuleType("antenv.axon_hooks")
    _m._HOOK = None
    _m.set_axon_ntff_profile_hook = lambda h: setattr(_m, "_HOOK", h)
    _m.get_axon_ntff_profile_hook = lambda: _m._HOOK
    sys.modules["antenv.axon_hooks"] = _m
    try:
        import antenv

        antenv.axon_hooks = _m
        from trn_agent_boot.trn_boot import _ntff_profile_via_ctypes

        _m.set_axon_ntff_profile_hook(
            _ntff_profile_via_ctypes("/opt/axon/libaxon_pjrt.so")
        )
    except Exception:
        pass

import bass_rust
import concourse.bass as bass
import concourse.tile as tile
from concourse import mybir
from concourse.bass_utils import run_bass_kernel_spmd
from contextlib import ExitStack

# ---------------------------------------------------------------------------
# Workaround for this walrus build's cap of ONE sync-wait command per
# instruction: Tile's sem-assignment freely attaches several waits to one
# instruction and codegen rejects it ("Too many sync wait commands").
# Split the waits across same-engine NoOps preceding the instruction.
# ---------------------------------------------------------------------------
MAX_WAITS = 1


def split_multi_waits(nc):
    n_split = 0
    for f in nc.m.functions:
        for bb in f.blocks:
            out = []
            for inst in bb.instructions:
                si = inst.sync_info
                if si is not None and si.on_wait and len(si.on_wait) > MAX_WAITS:
                    waits = list(si.on_wait)
                    extra, keep = waits[:-MAX_WAITS], waits[-MAX_WAITS:]
                    for i in range(0, len(extra), MAX_WAITS):
                        nop = bass_rust.InstNoOp(
                            name=f"I-{nc.next_id()}", ins=[], outs=[]
                        )
                        nop.engine = inst.engine
                        nop.sync_info = mybir.SyncInfo(
                            on_wait=extra[i : i + MAX_WAITS], on_update=[]
                        )
                        out.append(nop)
                    si.on_wait = keep
                    n_split += 1
                out.append(inst)
            bb.instructions[:] = out
    return n_split



BF16 = mybir.dt.bfloat16
F32 = mybir.dt.float32

N_CORES = 8
B, S, HID = 2, 2048, 4096
BS = B * S  # 4096
D = 128
NH = 4          # q heads per core
KT = HID // 128  # 32 k-tiles
SC = 512        # free-dim chunk
NSC = BS // SC  # 8
SCALE = 1.0 / (D ** 0.5)
EXP = mybir.ActivationFunctionType.Exp
LOG = mybir.ActivationFunctionType.Ln


def build():
    nc = bass.Bass(num_devices=N_CORES)

    xT = nc.declare_dram_parameter("xT", [HID, BS], BF16, isOutput=False)
    wq = nc.declare_dram_parameter("wq", [HID, NH * D], BF16, isOutput=False)
    wk = nc.declare_dram_parameter("wk", [HID, D], BF16, isOutput=False)
    wv = nc.declare_dram_parameter("wv", [HID, D], BF16, isOutput=False)
    wo = nc.declare_dram_parameter("wo", [512, HID], BF16, isOutput=False)
    cosF = nc.declare_dram_parameter("cosF", [D, BS], BF16, isOutput=False)
    sinF = nc.declare_dram_parameter("sinF", [D, BS], BF16, isOutput=False)
    swapP = nc.declare_dram_parameter("swapP", [D, D], BF16, isOutput=False)
    masks = nc.declare_dram_parameter("masks", [D, 4 * SC], BF16, isOutput=False)
    outT = nc.declare_dram_parameter("outT", [HID, BS], BF16, isOutput=True)

    vT_d = nc.dram_tensor("vT_d", [D, BS], BF16)

    with tile.TileContext(nc, num_cores=N_CORES) as tc, ExitStack() as ctx:
        # ---- long-lived pools -------------------------------------------
        singles = ctx.enter_context(tc.tile_pool(name="singles", bufs=1))
        qkv_sb = ctx.enter_context(tc.tile_pool(name="qkv_sb", bufs=1))
        ps_acc = ctx.enter_context(tc.tile_pool(name="ps_acc", bufs=3, space="PSUM"))
        ps_s = ctx.enter_context(tc.tile_pool(name="ps_s", bufs=3, space="PSUM"))
        ps_l = ctx.enter_context(tc.tile_pool(name="ps_l", bufs=2, space="PSUM"))
        cos_sb = singles.tile([D, BS], BF16)
        nc.sync.dma_start(cos_sb[:], cosF[:])
        sin_sb = singles.tile([D, BS], BF16)
        nc.sync.dma_start(sin_sb[:], sinF[:])
        mask_sb = singles.tile([D, 4 * SC], BF16)
        nc.sync.dma_start(mask_sb[:], masks[:])
        swap_sb = singles.tile([D, D], BF16)
        nc.sync.dma_start(swap_sb[:], swapP[:])
        ones_sb = singles.tile([D, D], BF16)
        nc.vector.memset(ones_sb[:], 1.0)

        q_sb = [
            qkv_sb.tile([D, BS], BF16, tag=f"q{h}", name=f"q_sb{h}")
            for h in range(NH)
        ]
        k_sb = qkv_sb.tile([D, BS], BF16, tag="k")
        v_sb = qkv_sb.tile([D, KT, D], BF16, tag="v")  # V natural: [sk_local, j, d]

        # ---- phase 1: projections + rope --------------------------------
        with tc.tile_pool(name="w1", bufs=1) as w1, \
             tc.tile_pool(name="xt", bufs=3) as xtp, \
             tc.tile_pool(name="rope", bufs=4) as rope, \
             tc.tile_pool(name="vt", bufs=3) as vtp:

            wq_sb = w1.tile([128, KT, NH * D], BF16, tag="wq")
            nc.sync.dma_start(wq_sb[:], wq.rearrange("(k p) c -> p k c", p=128))
            wk_sb = w1.tile([128, KT, D], BF16, tag="wk")
            nc.sync.dma_start(wk_sb[:], wk.rearrange("(k p) c -> p k c", p=128))
            wv_sb = w1.tile([128, KT, D], BF16, tag="wv")
            nc.sync.dma_start(wv_sb[:], wv.rearrange("(k p) c -> p k c", p=128))

            def rope_a(ps_q):
                qeo = rope.tile([D, SC], BF16, tag="qeo")
                nc.vector.tensor_copy(qeo[:], ps_q[:])
                return qeo

            def rope_b(qeo, dst, sc):
                cols = bass.ts(sc, SC)
                ps_sw = ps_l.tile([D, SC], F32, tag="l")
                nc.tensor.matmul(ps_sw[:], swap_sb[:], qeo[:], start=True, stop=True)
                t1 = rope.tile([D, SC], BF16, tag="t1")
                nc.vector.tensor_mul(t1[:], qeo[:], cos_sb[:, cols])
                t2 = rope.tile([D, SC], BF16, tag="t2")
                nc.vector.tensor_mul(t2[:], ps_sw[:], sin_sb[:, cols])
                nc.vector.tensor_add(dst[:, cols], t1[:], t2[:])

            for sc in range(NSC):
                xth = []
                for g in range(2):
                    t = xtp.tile([128, KT // 2, SC], BF16, tag="xt")
                    nc.sync.dma_start(
                        t[:],
                        xT.rearrange("(k p) s -> p k s", p=128)[
                            :, g * (KT // 2):(g + 1) * (KT // 2), bass.ts(sc, SC)
                        ],
                    )
                    xth.append(t)
                xts = [xth[k // (KT // 2)][:, k % (KT // 2), :] for k in range(KT)]

                pending = None  # deferred rope_b so PE never waits on DVE copy
                for t_i in range(6):
                    ps_t = ps_acc.tile([D, SC], F32, tag="acc", name=f"ps_t{sc}_{t_i}")
                    for k in range(KT):
                        if t_i < NH:
                            lhs = wq_sb[:, k, bass.ts(t_i, D)]
                        elif t_i == NH:
                            lhs = wk_sb[:, k, :]
                        else:
                            lhs = wv_sb[:, k, :]
                        nc.tensor.matmul(
                            ps_t[:], lhs, xts[k],
                            start=(k == 0), stop=(k == KT - 1),
                        )
                    if t_i < 5:
                        qeo = rope_a(ps_t)
                        if pending is not None:
                            rope_b(*pending)
                        dst = q_sb[t_i] if t_i < NH else k_sb
                        pending = (qeo, dst, sc)
                    else:
                        vt = vtp.tile([D, SC], BF16, tag="vt")
                        nc.vector.tensor_copy(vt[:], ps_t[:])
                        nc.sync.dma_start(vT_d[:, bass.ts(sc, SC)], vt[:])
                        rope_b(*pending)

            # V: read back transposed -> natural (sk, d) tiles
            for j in range(KT):
                nc.sync.dma_start_transpose(
                    v_sb[:, j, :], vT_d[:, bass.ts(j, D)]
                )

        # ---- phase 3 emitter (called per batch half) --------------------
        # No collective: each core contracts only its own 4 heads' A^T
        # (512 of 4096 rows) against its w_o row-slice, producing a full
        # (HID, BS) partial that the host sums across cores.
        def wo_pools(i):
            return [(ps_acc, "acc"), (ps_s, "s"), (ps_l, "l")][i % 3]

        def emit_wo_half(b, a_all):
            for nl in range(S // SC):
                col = b * S + nl * SC
                for m in range(KT):
                    pool, tg = wo_pools(m)
                    o_ps = pool.tile([D, SC], F32, tag=tg, name=f"wo{b}_{nl}_{m}")
                    for h in range(NH):
                        nc.tensor.matmul(
                            o_ps[:],
                            wo_sb[:, h, bass.ts(m, D)],
                            a_all[h][:, nl * SC:(nl + 1) * SC],
                            start=(h == 0), stop=(h == NH - 1),
                        )
                    ot = o3p.tile([D, SC], BF16, tag="ot", name=f"ot{b}_{nl}_{m}")
                    nc.vector.tensor_copy(ot[:], o_ps[:])
                    nc.sync.dma_start(
                        outT[bass.ts(m, D), col:col + SC], ot[:]
                    )

        # ---- phase 2: attention -----------------------------------------
        with tc.tile_pool(name="pexp", bufs=6) as pexp, \
             tc.tile_pool(name="asml", bufs=4) as asml, \
             tc.tile_pool(name="w3", bufs=1) as w3, \
             tc.tile_pool(name="aall", bufs=2) as aallp, \
             tc.tile_pool(name="o3p", bufs=4) as o3p:

            wo_sb = w3.tile([128, NH, HID], BF16, tag="wo")
            nc.gpsimd.dma_start(wo_sb[:], wo.rearrange("(k p) c -> p k c", p=128))

            for b in range(B):
                a_all = [
                    aallp.tile([D, S], BF16, tag=f"a{h}", name=f"a_all{b}_{h}")
                    for h in range(NH)
                ]
                for h in range(NH):
                    qh = q_sb[h]
                    for c in range(S // SC):  # 4 sq chunks per batch
                        sq = b * S + c * SC
                        nsk = 4 * (c + 1)
                        l_ps = ps_l.tile([D, SC], F32, tag="l")
                        o_ps = ps_acc.tile([D, SC], F32, tag="acc")
                        pend = []  # up to 2 tiles of PE lookahead

                        def flush(stop, pend=pend, l_ps=l_ps, o_ps=o_ps, b=b):
                            jp, pp = pend.pop(0)
                            nc.tensor.matmul(
                                l_ps[:], ones_sb[:], pp[:],
                                start=(jp == 0), stop=stop,
                            )
                            nc.tensor.matmul(
                                o_ps[:], v_sb[:, b * (S // D) + jp, :], pp[:],
                                start=(jp == 0), stop=stop,
                            )

                        for j in range(nsk):
                            s_ps = ps_s.tile([D, SC], F32, tag="s")
                            nc.tensor.matmul(
                                s_ps[:],
                                k_sb[:, b * S + j * D: b * S + (j + 1) * D],
                                qh[:, sq:sq + SC],
                                start=True, stop=True,
                            )
                            if len(pend) == 2:
                                flush(False)
                            p_sb = pexp.tile([D, SC], BF16, tag="p")
                            nc.scalar.activation(p_sb[:], s_ps[:], EXP, scale=SCALE)
                            dd = j - 4 * c
                            if dd >= 0:
                                nc.vector.tensor_mul(
                                    p_sb[:], p_sb[:], mask_sb[:, bass.ts(dd, SC)]
                                )
                            pend.append((j, p_sb))
                        while pend:
                            flush(len(pend) == 1)
                        # 1/l = exp(-ln(l)); ACT reciprocal is banned.
                        lg = asml.tile([D, SC], F32, tag="lg")
                        nc.scalar.activation(lg[:], l_ps[:], LOG)
                        rec = asml.tile([D, SC], F32, tag="rec")
                        nc.scalar.activation(rec[:], lg[:], EXP, scale=-1.0)
                        nc.vector.tensor_mul(
                            a_all[h][:, c * SC:(c + 1) * SC], o_ps[:], rec[:]
                        )
                # this batch half's output projection fills in right after
                emit_wo_half(b, a_all)

    split_multi_waits(nc)
    return nc


BF16_NP = ml_dtypes.bfloat16


def prep_inputs(x, cos_half, sin_half, w_q, w_k, w_v, w_o):
    x = np.asarray(x)
    cos_half = np.asarray(cos_half, dtype=np.float32)
    sin_half = np.asarray(sin_half, dtype=np.float32)
    w_q, w_k, w_v, w_o = (np.asarray(a) for a in (w_q, w_k, w_v, w_o))

    X = x.reshape(B * S, HID)
    xT = np.ascontiguousarray(X.T)  # (HID, BS) bf16

    cosb = cos_half.astype(BF16_NP)  # reference casts cos/sin to bf16 in _rope
    sinb = sin_half.astype(BF16_NP)
    cosF = np.tile(np.repeat(cosb.T, 2, axis=0), (1, B))  # (128, BS)
    sign = np.where(np.arange(D) % 2 == 0, -1.0, 1.0).astype(np.float32)
    sinF = np.tile(np.repeat(sinb.T, 2, axis=0) * sign[:, None].astype(BF16_NP), (1, B))
    cosF = np.ascontiguousarray(cosF, dtype=BF16_NP)
    sinF = np.ascontiguousarray(sinF, dtype=BF16_NP)

    swapP = np.zeros((D, D), dtype=BF16_NP)
    for i in range(D):
        swapP[i, i ^ 1] = 1.0

    # masks[p, dd*512 + f] = 1 if f >= p + 128*dd  (diagonal tile dd)
    p = np.arange(D)[:, None]
    f = np.arange(512)[None, :]
    masks = np.concatenate(
        [(f >= p + 128 * dd) for dd in range(4)], axis=1
    ).astype(BF16_NP)


    in_maps = []
    for c in range(N_CORES):
        in_maps.append(
            {
                "xT": xT,
                "wq": np.ascontiguousarray(w_q[:, c * 512:(c + 1) * 512]),
                "wk": np.ascontiguousarray(w_k[:, c * D:(c + 1) * D]),
                "wv": np.ascontiguousarray(w_v[:, c * D:(c + 1) * D]),
                "wo": np.ascontiguousarray(w_o[c * 512:(c + 1) * 512, :]),
                "cosF": cosF,
                "sinF": sinF,
                "swapP": swapP,
                "masks": masks,
            }
        )
    return in_maps


def kernel(x, cos_half, sin_half, w_q, w_k, w_v, w_o, trace=None):
    if trace is None:
        trace = os.environ.get("KTRACE", "0") == "1"
    global LAST_RESULT
    in_maps = prep_inputs(x, cos_half, sin_half, w_q, w_k, w_v, w_o)
    res = run_bass_kernel_spmd(
        _nc(), in_maps, core_ids=list(range(N_CORES)), trace=trace
    )
    LAST_RESULT = res
    acc = res.results[0]["outT"].astype(np.float32)
    for c in range(1, N_CORES):
        acc += res.results[c]["outT"].astype(np.float32)
    return np.ascontiguousarray(acc.T).astype(BF16_NP).reshape(B, S, HID)


_NC = None
LAST_RESULT = None


def _nc():
    global _NC
    if _NC is None:
        _NC = build()
    return _NC



# revision 7
# speedup vs baseline: 1.0043x; 1.0043x over previous
"""Trainium2 Bass kernel for nn_Attention_27994596836196.

GQA attention block (B=2, S=2048, HID=4096, 32 q heads / 8 kv groups,
rope, causal, out-projection), tensor-parallel over the 8 NeuronCores of
one TRN2 chip: core c owns q heads 4c..4c+3 and kv group c.  Inside the
device kernel each core computes its heads' Q^T/K^T/V projections from a
host-pretransposed activation matrix, runs causal flash-style attention
in a transposed (keys-on-partitions) layout, and contracts its 512-row
slice of w_o into a full-size partial output; the host sums the eight
partials (collectives deliberately avoided: a collective in the NEFF
measurably slows every PE instruction by ~20%).

Self-contained: builds and runs via concourse (bass/tile) from
/opt/trn_rl_repo through bass_utils.run_bass_kernel_spmd on cores 0-7.
"""

import os
import sys

sys.path.insert(0, "/opt/trn_rl_repo")

import numpy as np
import ml_dtypes

# NTFF profiling hook shim: this agent image's antenv package lacks
# axon_hooks, which run_bass_kernel_spmd(trace=True) imports.  Harmless
# when tracing is off; registers the real hook when available.
try:
    import antenv.axon_hooks  # noqa: F401
except ImportError:
    import types

    _m = types.ModuleType("antenv.axon_hooks")
    _m._HOOK = None
    _m.set_axon_ntff_profile_hook = lambda h: setattr(_m, "_HOOK", h)
    _m.get_axon_ntff_profile_hook = lambda: _m._HOOK
    sys.modules["antenv.axon_hooks"] = _m
    try:
        import antenv

        antenv.axon_hooks = _m
        from trn_agent_boot.trn_boot import _ntff_profile_via_ctypes

        _m.set_axon_ntff_profile_hook(
            _ntff_profile_via_ctypes("/opt/axon/libaxon_pjrt.so")
        )
    except Exception:
        pass

import bass_rust
import concourse.bass as bass
import concourse.tile as tile
from concourse import mybir
from concourse.bass_utils import run_bass_kernel_spmd
from contextlib import ExitStack

# ---------------------------------------------------------------------------
# Workaround for this walrus build's cap of ONE sync-wait command per
# instruction: Tile's sem-assignment freely attaches several waits to one
# instruction and codegen rejects it ("Too many sync wait commands").
# Split the waits across same-engine NoOps preceding the instruction.
# ---------------------------------------------------------------------------
MAX_WAITS = 1


def split_multi_waits(nc):
    n_split = 0
    for f in nc.m.functions:
        for bb in f.blocks:
            out = []
            for inst in bb.instructions:
                si = inst.sync_info
                if si is not None and si.on_wait and len(si.on_wait) > MAX_WAITS:
                    waits = list(si.on_wait)
                    extra, keep = waits[:-MAX_WAITS], waits[-MAX_WAITS:]
                    for i in range(0, len(extra), MAX_WAITS):
                        nop = bass_rust.InstNoOp(
                            name=f"I-{nc.next_id()}", ins=[], outs=[]
                        )
                        nop.engine = inst.engine
                        nop.sync_info = mybir.SyncInfo(
                            on_wait=extra[i : i + MAX_WAITS], on_update=[]
                        )
                        out.append(nop)
                    si.on_wait = keep
                    n_split += 1
                out.append(inst)
            bb.instructions[:] = out
    return n_split



BF16 = mybir.dt.bfloat16
F32 = mybir.dt.float32

N_CORES = 8
B, S, HID = 2, 2048, 4096
BS = B * S  # 4096
D = 128
NH = 4          # q heads per core
KT = HID // 128  # 32 k-tiles
SC = 512        # free-dim chunk
NSC = BS // SC  # 8
SCALE = 1.0 / (D ** 0.5)
EXP = mybir.ActivationFunctionType.Exp
LOG = mybir.ActivationFunctionType.Ln


def build():
    nc = bass.Bass(num_devices=N_CORES)

    # Host-prepped layouts: partition dim first, per-partition contiguous
    # blocks so every HWDGE DMA uses >=1KB descriptors.
    xTc = nc.declare_dram_parameter("xTc", [128, NSC, KT, SC], BF16, isOutput=False)
    wq = nc.declare_dram_parameter("wq", [128, KT, NH * D], BF16, isOutput=False)
    wk = nc.declare_dram_parameter("wk", [128, KT, D], BF16, isOutput=False)
    wv = nc.declare_dram_parameter("wv", [128, KT, D], BF16, isOutput=False)
    wo = nc.declare_dram_parameter("wo", [128, NH, HID], BF16, isOutput=False)
    cosF = nc.declare_dram_parameter("cosF", [D, BS], BF16, isOutput=False)
    sinF = nc.declare_dram_parameter("sinF", [D, BS], BF16, isOutput=False)
    swapP = nc.declare_dram_parameter("swapP", [D, D], BF16, isOutput=False)
    masks = nc.declare_dram_parameter("masks", [D, 4 * SC], BF16, isOutput=False)
    outT = nc.declare_dram_parameter("outT", [HID, BS], BF16, isOutput=True)

    vT_d = nc.dram_tensor("vT_d", [D, BS], BF16)

    with tile.TileContext(nc, num_cores=N_CORES) as tc, ExitStack() as ctx:
        # ---- long-lived pools -------------------------------------------
        singles = ctx.enter_context(tc.tile_pool(name="singles", bufs=1))
        qkv_sb = ctx.enter_context(tc.tile_pool(name="qkv_sb", bufs=1))
        ps_acc = ctx.enter_context(tc.tile_pool(name="ps_acc", bufs=3, space="PSUM"))
        ps_s = ctx.enter_context(tc.tile_pool(name="ps_s", bufs=3, space="PSUM"))
        ps_l = ctx.enter_context(tc.tile_pool(name="ps_l", bufs=2, space="PSUM"))

        q_sb = [
            qkv_sb.tile([D, BS], BF16, tag=f"q{h}", name=f"q_sb{h}")
            for h in range(NH)
        ]
        k_sb = qkv_sb.tile([D, BS], BF16, tag="k")
        v_sb = qkv_sb.tile([D, KT, D], BF16, tag="v")  # V natural: [sk_local, j, d]

        # ---- phase 1: projections + rope --------------------------------
        with tc.tile_pool(name="w1", bufs=1) as w1, \
             tc.tile_pool(name="xt", bufs=4) as xtp, \
             tc.tile_pool(name="rope", bufs=4) as rope, \
             tc.tile_pool(name="vt", bufs=3) as vtp:

            # scalar HWDGE queue: wq in 4 k-chunks so t_i=0 can start once
            # chunk 0 + the matching xt slices have landed.
            wq_sb = w1.tile([128, KT, NH * D], BF16, tag="wq")
            for g in range(4):
                nc.scalar.dma_start(
                    wq_sb[:, g * 8:(g + 1) * 8, :], wq[:, g * 8:(g + 1) * 8, :]
                )
            # gpsimd SWDGE queue: k/v weights (needed from t_i=4), then rope
            # tables (needed ~10us in), then the attention masks (phase 2).
            wk_sb = w1.tile([128, KT, D], BF16, tag="wk")
            nc.gpsimd.dma_start(wk_sb[:], wk[:])
            wv_sb = w1.tile([128, KT, D], BF16, tag="wv")
            nc.gpsimd.dma_start(wv_sb[:], wv[:])
            cos_sb = singles.tile([D, BS], BF16)
            nc.gpsimd.dma_start(cos_sb[:], cosF[:])
            sin_sb = singles.tile([D, BS], BF16)
            nc.gpsimd.dma_start(sin_sb[:], sinF[:])
            swap_sb = singles.tile([D, D], BF16)
            nc.gpsimd.dma_start(swap_sb[:], swapP[:])
            mask_sb = singles.tile([D, 4 * SC], BF16)
            nc.gpsimd.dma_start(mask_sb[:], masks[:])
            ones_sb = singles.tile([D, D], BF16)
            nc.vector.memset(ones_sb[:], 1.0)

            def rope_a(ps_q):
                qeo = rope.tile([D, SC], BF16, tag="qeo")
                nc.vector.tensor_copy(qeo[:], ps_q[:])
                return qeo

            def rope_b(qeo, dst, sc):
                cols = bass.ts(sc, SC)
                ps_sw = ps_l.tile([D, SC], F32, tag="l")
                nc.tensor.matmul(ps_sw[:], swap_sb[:], qeo[:], start=True, stop=True)
                t1 = rope.tile([D, SC], BF16, tag="t1")
                nc.vector.tensor_mul(t1[:], qeo[:], cos_sb[:, cols])
                t2 = rope.tile([D, SC], BF16, tag="t2")
                nc.vector.tensor_mul(t2[:], ps_sw[:], sin_sb[:, cols])
                nc.vector.tensor_add(dst[:, cols], t1[:], t2[:])

            for sc in range(NSC):
                xth = []
                for g in range(2):
                    t = xtp.tile([128, KT // 2, SC], BF16, tag="xt")
                    src = xTc[:, sc, g * (KT // 2):(g + 1) * (KT // 2), :]
                    if sc == 0:
                        # fine-grained arrival so the k-loop can start early
                        for q4 in range(4):
                            nc.sync.dma_start(
                                t[:, q4 * 4:(q4 + 1) * 4, :],
                                xTc[:, 0,
                                    g * (KT // 2) + q4 * 4:
                                    g * (KT // 2) + (q4 + 1) * 4, :],
                            )
                    else:
                        nc.sync.dma_start(t[:], src)
                    xth.append(t)
                xts = [xth[k // (KT // 2)][:, k % (KT // 2), :] for k in range(KT)]

                pending = None  # deferred rope_b so PE never waits on DVE copy
                for t_i in range(6):
                    ps_t = ps_acc.tile([D, SC], F32, tag="acc", name=f"ps_t{sc}_{t_i}")
                    for k in range(KT):
                        if t_i < NH:
                            lhs = wq_sb[:, k, bass.ts(t_i, D)]
                        elif t_i == NH:
                            lhs = wk_sb[:, k, :]
                        else:
                            lhs = wv_sb[:, k, :]
                        nc.tensor.matmul(
                            ps_t[:], lhs, xts[k],
                            start=(k == 0), stop=(k == KT - 1),
                        )
                    if t_i < 5:
                        qeo = rope_a(ps_t)
                        if pending is not None:
                            rope_b(*pending)
                        dst = q_sb[t_i] if t_i < NH else k_sb
                        pending = (qeo, dst, sc)
                    else:
                        vt = vtp.tile([D, SC], BF16, tag="vt")
                        nc.vector.tensor_copy(vt[:], ps_t[:])
                        nc.scalar.dma_start(vT_d[:, bass.ts(sc, SC)], vt[:])
                        rope_b(*pending)

                # V: read this chunk back transposed -> natural (sk, d) tiles
                # (scalar HWDGE queue; overlaps the remaining chunks' compute)
                for j in range(4 * sc, 4 * (sc + 1)):
                    nc.scalar.dma_start_transpose(
                        v_sb[:, j, :], vT_d[:, bass.ts(j, D)]
                    )

        # ---- phase 3 emitter (called per batch half) --------------------
        # No collective: each core contracts only its own 4 heads' A^T
        # (512 of 4096 rows) against its w_o row-slice, producing a full
        # (HID, BS) partial that the host sums across cores.
        def wo_pools(i):
            return [(ps_acc, "acc"), (ps_s, "s"), (ps_l, "l")][i % 3]

        def emit_wo_half(b, a_all):
            for nl in range(S // SC):
                col = b * S + nl * SC
                for m in range(KT):
                    pool, tg = wo_pools(m)
                    o_ps = pool.tile([D, SC], F32, tag=tg, name=f"wo{b}_{nl}_{m}")
                    for h in range(NH):
                        nc.tensor.matmul(
                            o_ps[:],
                            wo_sb[:, h, bass.ts(m, D)],
                            a_all[h][:, nl * SC:(nl + 1) * SC],
                            start=(h == 0), stop=(h == NH - 1),
                        )
                    ot = o3p.tile([D, SC], BF16, tag="ot", name=f"ot{b}_{nl}_{m}")
                    nc.vector.tensor_copy(ot[:], o_ps[:])
                    nc.sync.dma_start(
                        outT[bass.ts(m, D), col:col + SC], ot[:]
                    )

        # ---- phase 2: attention -----------------------------------------
        with tc.tile_pool(name="pexp", bufs=6) as pexp, \
             tc.tile_pool(name="asml", bufs=4) as asml, \
             tc.tile_pool(name="w3", bufs=1) as w3, \
             tc.tile_pool(name="aall", bufs=2) as aallp, \
             tc.tile_pool(name="o3p", bufs=4) as o3p:

            wo_sb = w3.tile([128, NH, HID], BF16, tag="wo")
            for g in range(4):
                nc.gpsimd.dma_start(
                    wo_sb[:, :, g * (HID // 4):(g + 1) * (HID // 4)],
                    wo[:, :, g * (HID // 4):(g + 1) * (HID // 4)],
                )

            for b in range(B):
                a_all = [
                    aallp.tile([D, S], BF16, tag=f"a{h}", name=f"a_all{b}_{h}")
                    for h in range(NH)
                ]
                for h in range(NH):
                    qh = q_sb[h]
                    for c in range(S // SC):  # 4 sq chunks per batch
                        sq = b * S + c * SC
                        nsk = 4 * (c + 1)
                        l_ps = ps_l.tile([D, SC], F32, tag="l")
                        o_ps = ps_acc.tile([D, SC], F32, tag="acc")
                        pend = []  # up to 2 tiles of PE lookahead

                        def flush(stop, pend=pend, l_ps=l_ps, o_ps=o_ps, b=b):
                            jp, pp = pend.pop(0)
                            nc.tensor.matmul(
                                l_ps[:], ones_sb[:], pp[:],
                                start=(jp == 0), stop=stop,
                            )
                            nc.tensor.matmul(
                                o_ps[:], v_sb[:, b * (S // D) + jp, :], pp[:],
                                start=(jp == 0), stop=stop,
                            )

                        for j in range(nsk):
                            s_ps = ps_s.tile([D, SC], F32, tag="s")
                            nc.tensor.matmul(
                                s_ps[:],
                                k_sb[:, b * S + j * D: b * S + (j + 1) * D],
                                qh[:, sq:sq + SC],
                                start=True, stop=True,
                            )
                            if len(pend) == 2:
                                flush(False)
                            p_sb = pexp.tile([D, SC], BF16, tag="p")
                            nc.scalar.activation(p_sb[:], s_ps[:], EXP, scale=SCALE)
                            dd = j - 4 * c
                            if dd >= 0:
                                nc.vector.tensor_mul(
                                    p_sb[:], p_sb[:], mask_sb[:, bass.ts(dd, SC)]
                                )
                            pend.append((j, p_sb))
                        while pend:
                            flush(len(pend) == 1)
                        # 1/l = exp(-ln(l)); ACT reciprocal is banned.
                        lg = asml.tile([D, SC], F32, tag="lg")
                        nc.scalar.activation(lg[:], l_ps[:], LOG)
                        rec = asml.tile([D, SC], F32, tag="rec")
                        nc.scalar.activation(rec[:], lg[:], EXP, scale=-1.0)
                        nc.vector.tensor_mul(
                            a_all[h][:, c * SC:(c + 1) * SC], o_ps[:], rec[:]
                        )
                # this batch half's output projection fills in right after
                emit_wo_half(b, a_all)

    split_multi_waits(nc)
    return nc


BF16_NP = ml_dtypes.bfloat16


def prep_inputs(x, cos_half, sin_half, w_q, w_k, w_v, w_o):
    x = np.asarray(x)
    cos_half = np.asarray(cos_half, dtype=np.float32)
    sin_half = np.asarray(sin_half, dtype=np.float32)
    w_q, w_k, w_v, w_o = (np.asarray(a) for a in (w_q, w_k, w_v, w_o))

    X = x.reshape(B * S, HID)
    xT = np.ascontiguousarray(X.T)  # (HID, BS) bf16
    # chunk-major layout: xTc[p, sc, k, s] = xT[p + 128k, sc*512 + s] so a
    # phase-1 tile DMA reads one contiguous 16-32KB block per partition
    xTc = np.ascontiguousarray(
        xT.reshape(KT, 128, NSC, SC).transpose(1, 2, 0, 3)
    )

    cosb = cos_half.astype(BF16_NP)  # reference casts cos/sin to bf16 in _rope
    sinb = sin_half.astype(BF16_NP)
    cosF = np.tile(np.repeat(cosb.T, 2, axis=0), (1, B))  # (128, BS)
    sign = np.where(np.arange(D) % 2 == 0, -1.0, 1.0).astype(np.float32)
    sinF = np.tile(np.repeat(sinb.T, 2, axis=0) * sign[:, None].astype(BF16_NP), (1, B))
    cosF = np.ascontiguousarray(cosF, dtype=BF16_NP)
    sinF = np.ascontiguousarray(sinF, dtype=BF16_NP)

    swapP = np.zeros((D, D), dtype=BF16_NP)
    for i in range(D):
        swapP[i, i ^ 1] = 1.0

    # masks[p, dd*512 + f] = 1 if f >= p + 128*dd  (diagonal tile dd)
    p = np.arange(D)[:, None]
    f = np.arange(512)[None, :]
    masks = np.concatenate(
        [(f >= p + 128 * dd) for dd in range(4)], axis=1
    ).astype(BF16_NP)


    def pmajor(w):  # (4096, C) -> (128, KT_w, C) with row r = p + 128k
        kt = w.shape[0] // 128
        return np.ascontiguousarray(w.reshape(kt, 128, w.shape[1]).transpose(1, 0, 2))

    in_maps = []
    for c in range(N_CORES):
        in_maps.append(
            {
                "xTc": xTc,
                "wq": pmajor(w_q[:, c * 512:(c + 1) * 512]),
                "wk": pmajor(w_k[:, c * D:(c + 1) * D]),
                "wv": pmajor(w_v[:, c * D:(c + 1) * D]),
                "wo": pmajor(w_o[c * 512:(c + 1) * 512, :]),
                "cosF": cosF,
                "sinF": sinF,
                "swapP": swapP,
                "masks": masks,
            }
        )
    return in_maps


def kernel(x, cos_half, sin_half, w_q, w_k, w_v, w_o, trace=None):
    if trace is None:
        trace = os.environ.get("KTRACE", "0") == "1"
    global LAST_RESULT
    in_maps = prep_inputs(x, cos_half, sin_half, w_q, w_k, w_v, w_o)
    res = run_bass_kernel_spmd(
        _nc(), in_maps, core_ids=list(range(N_CORES)), trace=trace
    )
    LAST_RESULT = res
    acc = res.results[0]["outT"].astype(np.float32)
    for c in range(1, N_CORES):
        acc += res.results[c]["outT"].astype(np.float32)
    return np.ascontiguousarray(acc.T).astype(BF16_NP).reshape(B, S, HID)


_NC = None
LAST_RESULT = None


def _nc():
    global _NC
    if _NC is None:
        _NC = build()
    return _NC



# revision 9
# speedup vs baseline: 1.0306x; 1.0261x over previous
"""Trainium2 Bass kernel for nn_Attention_27994596836196.

GQA attention block (B=2, S=2048, HID=4096, 32 q heads / 8 kv groups,
rope, causal, out-projection), tensor-parallel over the 8 NeuronCores of
one TRN2 chip: core c owns q heads 4c..4c+3 and kv group c.  Inside the
device kernel each core computes its heads' Q^T/K^T/V projections from a
host-pretransposed activation matrix, runs causal flash-style attention
in a transposed (keys-on-partitions) layout, and contracts its 512-row
slice of w_o into a full-size partial output; the host sums the eight
partials (collectives deliberately avoided: a collective in the NEFF
measurably slows every PE instruction by ~20%).

Self-contained: builds and runs via concourse (bass/tile) from
/opt/trn_rl_repo through bass_utils.run_bass_kernel_spmd on cores 0-7.
"""

import os
import sys

sys.path.insert(0, "/opt/trn_rl_repo")

import numpy as np
import ml_dtypes

# NTFF profiling hook shim: this agent image's antenv package lacks
# axon_hooks, which run_bass_kernel_spmd(trace=True) imports.  Harmless
# when tracing is off; registers the real hook when available.
try:
    import antenv.axon_hooks  # noqa: F401
except ImportError:
    import types

    _m = types.ModuleType("antenv.axon_hooks")
    _m._HOOK = None
    _m.set_axon_ntff_profile_hook = lambda h: setattr(_m, "_HOOK", h)
    _m.get_axon_ntff_profile_hook = lambda: _m._HOOK
    sys.modules["antenv.axon_hooks"] = _m
    try:
        import antenv

        antenv.axon_hooks = _m
        from trn_agent_boot.trn_boot import _ntff_profile_via_ctypes

        _m.set_axon_ntff_profile_hook(
            _ntff_profile_via_ctypes("/opt/axon/libaxon_pjrt.so")
        )
    except Exception:
        pass

import bass_rust
import concourse.bass as bass
import concourse.tile as tile
from concourse import mybir
from concourse.bass_utils import run_bass_kernel_spmd
from contextlib import ExitStack

# ---------------------------------------------------------------------------
# Workaround for this walrus build's cap of ONE sync-wait command per
# instruction: Tile's sem-assignment freely attaches several waits to one
# instruction and codegen rejects it ("Too many sync wait commands").
# Split the waits across same-engine NoOps preceding the instruction.
# ---------------------------------------------------------------------------
MAX_WAITS = 1


def split_multi_waits(nc):
    n_split = 0
    for f in nc.m.functions:
        for bb in f.blocks:
            out = []
            for inst in bb.instructions:
                si = inst.sync_info
                if si is not None and si.on_wait and len(si.on_wait) > MAX_WAITS:
                    waits = list(si.on_wait)
                    extra, keep = waits[:-MAX_WAITS], waits[-MAX_WAITS:]
                    for i in range(0, len(extra), MAX_WAITS):
                        nop = bass_rust.InstNoOp(
                            name=f"I-{nc.next_id()}", ins=[], outs=[]
                        )
                        nop.engine = inst.engine
                        nop.sync_info = mybir.SyncInfo(
                            on_wait=extra[i : i + MAX_WAITS], on_update=[]
                        )
                        out.append(nop)
                    si.on_wait = keep
                    n_split += 1
                out.append(inst)
            bb.instructions[:] = out
    return n_split



BF16 = mybir.dt.bfloat16
F32 = mybir.dt.float32

N_CORES = 8
B, S, HID = 2, 2048, 4096
BS = B * S  # 4096
D = 128
NH = 4          # q heads per core
KT = HID // 128  # 32 k-tiles
SC = 512        # free-dim chunk
NSC = BS // SC  # 8
SCALE = 1.0 / (D ** 0.5)
EXP = mybir.ActivationFunctionType.Exp
LOG = mybir.ActivationFunctionType.Ln


def build():
    nc = bass.Bass(num_devices=N_CORES)

    # Host-prepped layouts: partition dim first, per-partition contiguous
    # blocks so every HWDGE DMA uses >=1KB descriptors.
    xTc = nc.declare_dram_parameter("xTc", [128, NSC, KT, SC], BF16, isOutput=False)
    wq = nc.declare_dram_parameter("wq", [128, KT, NH * D], BF16, isOutput=False)
    wk = nc.declare_dram_parameter("wk", [128, KT, D], BF16, isOutput=False)
    wv = nc.declare_dram_parameter("wv", [128, KT, D], BF16, isOutput=False)
    wo = nc.declare_dram_parameter("wo", [128, NH, HID], BF16, isOutput=False)
    cosF = nc.declare_dram_parameter("cosF", [D, BS], BF16, isOutput=False)
    sinF = nc.declare_dram_parameter("sinF", [D, BS], BF16, isOutput=False)
    swapP = nc.declare_dram_parameter("swapP", [D, D], BF16, isOutput=False)
    masks = nc.declare_dram_parameter("masks", [D, 4 * SC], BF16, isOutput=False)
    outT = nc.declare_dram_parameter("outT", [HID, BS], BF16, isOutput=True)

    vT_d = nc.dram_tensor("vT_d", [D, BS], BF16)

    with tile.TileContext(nc, num_cores=N_CORES) as tc, ExitStack() as ctx:
        # ---- long-lived pools -------------------------------------------
        singles = ctx.enter_context(tc.tile_pool(name="singles", bufs=1))
        qkv_sb = ctx.enter_context(tc.tile_pool(name="qkv_sb", bufs=1))
        ps_acc = ctx.enter_context(tc.tile_pool(name="ps_acc", bufs=3, space="PSUM"))
        ps_s = ctx.enter_context(tc.tile_pool(name="ps_s", bufs=3, space="PSUM"))
        ps_l = ctx.enter_context(tc.tile_pool(name="ps_l", bufs=2, space="PSUM"))

        q_sb = [
            qkv_sb.tile([D, BS], BF16, tag=f"q{h}", name=f"q_sb{h}")
            for h in range(NH)
        ]
        k_sb = qkv_sb.tile([D, BS], BF16, tag="k")
        v_sb = qkv_sb.tile([D, KT, D], BF16, tag="v")  # V natural: [sk_local, j, d]

        # ---- phase 1: projections + rope --------------------------------
        with tc.tile_pool(name="w1", bufs=1) as w1, \
             tc.tile_pool(name="xt", bufs=4) as xtp, \
             tc.tile_pool(name="rope", bufs=4) as rope, \
             tc.tile_pool(name="vt", bufs=3) as vtp:

            # scalar HWDGE queue (idle engine in phase 1): weights in k-chunks,
            # in the order compute consumes them: wk (first projection), wv,
            # then the 4MB wq.
            wk_sb = w1.tile([128, KT, D], BF16, tag="wk")
            wv_sb = w1.tile([128, KT, D], BF16, tag="wv")
            wq_sb = w1.tile([128, KT, NH * D], BF16, tag="wq")
            for g in range(4):
                nc.scalar.dma_start(
                    wk_sb[:, g * 8:(g + 1) * 8, :], wk[:, g * 8:(g + 1) * 8, :]
                )
            for g in range(4):
                nc.scalar.dma_start(
                    wv_sb[:, g * 8:(g + 1) * 8, :], wv[:, g * 8:(g + 1) * 8, :]
                )
            for g in range(4):
                nc.scalar.dma_start(
                    wq_sb[:, g * 8:(g + 1) * 8, :], wq[:, g * 8:(g + 1) * 8, :]
                )
            # gpsimd SWDGE queue: rope tables (needed at the first rope_b,
            # which trails by a full t_i block) and the attention masks.
            cos_sb = singles.tile([D, BS], BF16)
            nc.gpsimd.dma_start(cos_sb[:], cosF[:])
            sin_sb = singles.tile([D, BS], BF16)
            nc.gpsimd.dma_start(sin_sb[:], sinF[:])
            swap_sb = singles.tile([D, D], BF16)
            nc.gpsimd.dma_start(swap_sb[:], swapP[:])
            mask_sb = singles.tile([D, 4 * SC], BF16)
            nc.gpsimd.dma_start(mask_sb[:], masks[:])
            ones_sb = singles.tile([D, D], BF16)
            nc.vector.memset(ones_sb[:], 1.0)

            def rope_a(ps_q):
                qeo = rope.tile([D, SC], BF16, tag="qeo")
                nc.vector.tensor_copy(qeo[:], ps_q[:])
                return qeo

            def rope_b(qeo, dst, sc):
                cols = bass.ts(sc, SC)
                ps_sw = ps_l.tile([D, SC], F32, tag="l")
                nc.tensor.matmul(ps_sw[:], swap_sb[:], qeo[:], start=True, stop=True)
                t1 = rope.tile([D, SC], BF16, tag="t1")
                nc.vector.tensor_mul(t1[:], qeo[:], cos_sb[:, cols])
                t2 = rope.tile([D, SC], BF16, tag="t2")
                nc.vector.tensor_mul(t2[:], ps_sw[:], sin_sb[:, cols])
                nc.vector.tensor_add(dst[:, cols], t1[:], t2[:])

            for sc in range(NSC):
                xth = []
                for g in range(2):
                    t = xtp.tile([128, KT // 2, SC], BF16, tag="xt")
                    src = xTc[:, sc, g * (KT // 2):(g + 1) * (KT // 2), :]
                    if sc == 0:
                        # fine-grained arrival so the k-loop can start early
                        for q4 in range(4):
                            nc.sync.dma_start(
                                t[:, q4 * 4:(q4 + 1) * 4, :],
                                xTc[:, 0,
                                    g * (KT // 2) + q4 * 4:
                                    g * (KT // 2) + (q4 + 1) * 4, :],
                            )
                    else:
                        nc.sync.dma_start(t[:], src)
                    xth.append(t)
                xts = [xth[k // (KT // 2)][:, k % (KT // 2), :] for k in range(KT)]

                # t_i order [k, v, q0..q3]: K first so compute starts on the
                # 1MB wk before the 4MB wq has streamed in.
                pending = None  # deferred rope_b so PE never waits on DVE copy
                for t_i in range(6):
                    ps_t = ps_acc.tile([D, SC], F32, tag="acc", name=f"ps_t{sc}_{t_i}")
                    for k in range(KT):
                        if t_i == 0:
                            lhs = wk_sb[:, k, :]
                        elif t_i == 1:
                            lhs = wv_sb[:, k, :]
                        else:
                            lhs = wq_sb[:, k, bass.ts(t_i - 2, D)]
                        nc.tensor.matmul(
                            ps_t[:], lhs, xts[k],
                            start=(k == 0), stop=(k == KT - 1),
                        )
                    if t_i == 1:
                        vt = vtp.tile([D, SC], BF16, tag="vt")
                        nc.vector.tensor_copy(vt[:], ps_t[:])
                        nc.sync.dma_start(vT_d[:, bass.ts(sc, SC)], vt[:])
                    else:
                        qeo = rope_a(ps_t)
                        if pending is not None:
                            rope_b(*pending)
                        dst = k_sb if t_i == 0 else q_sb[t_i - 2]
                        pending = (qeo, dst, sc)
                rope_b(*pending)

                # V: read this chunk back transposed -> natural (sk, d) tiles
                # (sync queue: scalar engine must stay clear for phase-2 exps)
                for j in range(4 * sc, 4 * (sc + 1)):
                    nc.sync.dma_start_transpose(
                        v_sb[:, j, :], vT_d[:, bass.ts(j, D)]
                    )

        # ---- phase 3 emitter (called per batch half) --------------------
        # No collective: each core contracts only its own 4 heads' A^T
        # (512 of 4096 rows) against its w_o row-slice, producing a full
        # (HID, BS) partial that the host sums across cores.
        def wo_pools(i):
            return [(ps_acc, "acc"), (ps_s, "s"), (ps_l, "l")][i % 3]

        def emit_wo_half(b, a_all):
            for nl in range(S // SC):
                col = b * S + nl * SC
                for m in range(KT):
                    pool, tg = wo_pools(m)
                    o_ps = pool.tile([D, SC], F32, tag=tg, name=f"wo{b}_{nl}_{m}")
                    for h in range(NH):
                        nc.tensor.matmul(
                            o_ps[:],
                            wo_sb[:, h, bass.ts(m, D)],
                            a_all[h][:, nl * SC:(nl + 1) * SC],
                            start=(h == 0), stop=(h == NH - 1),
                        )
                    ot = o3p.tile([D, SC], BF16, tag="ot", name=f"ot{b}_{nl}_{m}")
                    nc.vector.tensor_copy(ot[:], o_ps[:])
                    nc.sync.dma_start(
                        outT[bass.ts(m, D), col:col + SC], ot[:]
                    )

        # ---- phase 2: attention -----------------------------------------
        with tc.tile_pool(name="pexp", bufs=6) as pexp, \
             tc.tile_pool(name="asml", bufs=4) as asml, \
             tc.tile_pool(name="w3", bufs=1) as w3, \
             tc.tile_pool(name="aall", bufs=2) as aallp, \
             tc.tile_pool(name="o3p", bufs=4) as o3p:

            wo_sb = w3.tile([128, NH, HID], BF16, tag="wo")
            for g in range(4):
                nc.gpsimd.dma_start(
                    wo_sb[:, :, g * (HID // 4):(g + 1) * (HID // 4)],
                    wo[:, :, g * (HID // 4):(g + 1) * (HID // 4)],
                )

            for b in range(B):
                a_all = [
                    aallp.tile([D, S], BF16, tag=f"a{h}", name=f"a_all{b}_{h}")
                    for h in range(NH)
                ]
                for h in range(NH):
                    qh = q_sb[h]
                    for c in range(S // SC):  # 4 sq chunks per batch
                        sq = b * S + c * SC
                        nsk = 4 * (c + 1)
                        l_ps = ps_l.tile([D, SC], F32, tag="l")
                        o_ps = ps_acc.tile([D, SC], F32, tag="acc")
                        pend = []  # up to 2 tiles of PE lookahead

                        def flush(stop, pend=pend, l_ps=l_ps, o_ps=o_ps, b=b):
                            jp, pp = pend.pop(0)
                            nc.tensor.matmul(
                                l_ps[:], ones_sb[:], pp[:],
                                start=(jp == 0), stop=stop,
                            )
                            nc.tensor.matmul(
                                o_ps[:], v_sb[:, b * (S // D) + jp, :], pp[:],
                                start=(jp == 0), stop=stop,
                            )

                        for j in range(nsk):
                            s_ps = ps_s.tile([D, SC], F32, tag="s")
                            nc.tensor.matmul(
                                s_ps[:],
                                k_sb[:, b * S + j * D: b * S + (j + 1) * D],
                                qh[:, sq:sq + SC],
                                start=True, stop=True,
                            )
                            if len(pend) == 2:
                                flush(False)
                            p_sb = pexp.tile([D, SC], BF16, tag="p")
                            nc.scalar.activation(p_sb[:], s_ps[:], EXP, scale=SCALE)
                            dd = j - 4 * c
                            if dd >= 0:
                                nc.vector.tensor_mul(
                                    p_sb[:], p_sb[:], mask_sb[:, bass.ts(dd, SC)]
                                )
                            pend.append((j, p_sb))
                        while pend:
                            flush(len(pend) == 1)
                        # 1/l = exp(-ln(l)); ACT reciprocal is banned.
                        lg = asml.tile([D, SC], F32, tag="lg")
                        nc.scalar.activation(lg[:], l_ps[:], LOG)
                        rec = asml.tile([D, SC], F32, tag="rec")
                        nc.scalar.activation(rec[:], lg[:], EXP, scale=-1.0)
                        nc.vector.tensor_mul(
                            a_all[h][:, c * SC:(c + 1) * SC], o_ps[:], rec[:]
                        )
                # this batch half's output projection fills in right after
                emit_wo_half(b, a_all)

    split_multi_waits(nc)
    return nc


BF16_NP = ml_dtypes.bfloat16


def prep_inputs(x, cos_half, sin_half, w_q, w_k, w_v, w_o):
    x = np.asarray(x)
    cos_half = np.asarray(cos_half, dtype=np.float32)
    sin_half = np.asarray(sin_half, dtype=np.float32)
    w_q, w_k, w_v, w_o = (np.asarray(a) for a in (w_q, w_k, w_v, w_o))

    X = x.reshape(B * S, HID)
    xT = np.ascontiguousarray(X.T)  # (HID, BS) bf16
    # chunk-major layout: xTc[p, sc, k, s] = xT[p + 128k, sc*512 + s] so a
    # phase-1 tile DMA reads one contiguous 16-32KB block per partition
    xTc = np.ascontiguousarray(
        xT.reshape(KT, 128, NSC, SC).transpose(1, 2, 0, 3)
    )

    cosb = cos_half.astype(BF16_NP)  # reference casts cos/sin to bf16 in _rope
    sinb = sin_half.astype(BF16_NP)
    cosF = np.tile(np.repeat(cosb.T, 2, axis=0), (1, B))  # (128, BS)
    sign = np.where(np.arange(D) % 2 == 0, -1.0, 1.0).astype(np.float32)
    sinF = np.tile(np.repeat(sinb.T, 2, axis=0) * sign[:, None].astype(BF16_NP), (1, B))
    cosF = np.ascontiguousarray(cosF, dtype=BF16_NP)
    sinF = np.ascontiguousarray(sinF, dtype=BF16_NP)

    swapP = np.zeros((D, D), dtype=BF16_NP)
    for i in range(D):
        swapP[i, i ^ 1] = 1.0

    # masks[p, dd*512 + f] = 1 if f >= p + 128*dd  (diagonal tile dd)
    p = np.arange(D)[:, None]
    f = np.arange(512)[None, :]
    masks = np.concatenate(
        [(f >= p + 128 * dd) for dd in range(4)], axis=1
    ).astype(BF16_NP)


    def pmajor(w):  # (4096, C) -> (128, KT_w, C) with row r = p + 128k
        kt = w.shape[0] // 128
        return np.ascontiguousarray(w.reshape(kt, 128, w.shape[1]).transpose(1, 0, 2))

    in_maps = []
    for c in range(N_CORES):
        in_maps.append(
            {
                "xTc": xTc,
                "wq": pmajor(w_q[:, c * 512:(c + 1) * 512]),
                "wk": pmajor(w_k[:, c * D:(c + 1) * D]),
                "wv": pmajor(w_v[:, c * D:(c + 1) * D]),
                "wo": pmajor(w_o[c * 512:(c + 1) * 512, :]),
                "cosF": cosF,
                "sinF": sinF,
                "swapP": swapP,
                "masks": masks,
            }
        )
    return in_maps


def kernel(x, cos_half, sin_half, w_q, w_k, w_v, w_o, trace=None):
    if trace is None:
        trace = os.environ.get("KTRACE", "0") == "1"
    global LAST_RESULT
    in_maps = prep_inputs(x, cos_half, sin_half, w_q, w_k, w_v, w_o)
    res = run_bass_kernel_spmd(
        _nc(), in_maps, core_ids=list(range(N_CORES)), trace=trace
    )
    LAST_RESULT = res
    acc = res.results[0]["outT"].astype(np.float32)
    for c in range(1, N_CORES):
        acc += res.results[c]["outT"].astype(np.float32)
    return np.ascontiguousarray(acc.T).astype(BF16_NP).reshape(B, S, HID)


_NC = None
LAST_RESULT = None


def _nc():
    global _NC
    if _NC is None:
        _NC = build()
    return _NC

